# revision 1
# baseline (speedup 1.0000x reference)
"""Cross-attention layer (B=2, L=2048, D=1024, 16 heads) on 8 TRN2 NeuronCores.

Two-phase pipeline: phase 1 computes K^T / V projections sharded 8-way
over kv rows (no replication); host regathers per batch; phase 2 runs
Q-projection + attention + output projection + LayerNorm row-sharded.

Phase 1, core c (b = c//4, kv rows 512*(c%4)..):
    KT_part[hd, kv_slice] = (Wk^T kvT_slice) + bk,  V_part = kv_slice @ Wv
Phase 2, core c (b = c//4, q rows 512*(c%4)..): identical attention pipeline
to kernel.py but K^T / ones-augmented V arrive via DRAM instead of on-core
projection.
"""

import numpy as np

import concourse.mybir as mybir
import concourse.tile as tile
from concourse import bacc
from concourse.bass_utils import run_bass_kernel_spmd

dt = mybir.dt
AF = mybir.ActivationFunctionType
ALU = mybir.AluOpType

P = 128
B, LQ, LKV = 2, 2048, 2048
DQ, DKV, HID, NH = 1024, 1024, 1024, 16
HD = HID // NH
EPS = 1e-5
N_CORES = 8
RQ = LQ * B // N_CORES             # 512
RKV = LKV * B // N_CORES           # 512 kv rows per phase-1 core
KV_T = LKV // P                    # 16
DPO = DQ // P                      # 8
N_PAIR = NH // 2                   # 8
MQ = RQ // P                       # 4
VA = HD + 1                        # 65


def build_phase1():
    nc = bacc.Bacc("TRN2", target_bir_lowering=False, debug=False,
                   num_devices=N_CORES)
    f32r, f32 = dt.float32r, dt.float32
    kvTs_d = nc.dram_tensor("kvTs", [DKV, RKV], f32r, kind="ExternalInput")
    wk_d = nc.dram_tensor("wk", [DKV, HID], f32r, kind="ExternalInput")
    wv_d = nc.dram_tensor("wv", [DKV, HID], f32r, kind="ExternalInput")
    bk_d = nc.dram_tensor("bk", [P, DPO], f32, kind="ExternalInput")
    ktp_d = nc.dram_tensor("ktp", [HID, RKV], f32, kind="ExternalOutput")
    vp_d = nc.dram_tensor("vp", [RKV, HID], f32, kind="ExternalOutput")

    with tile.TileContext(nc) as tc:
        with (
            tc.tile_pool(name="c1", bufs=1) as c1,
            tc.tile_pool(name="wkp", bufs=8) as wkp,
            tc.tile_pool(name="wvp", bufs=3) as wvp,
            tc.tile_pool(name="op", bufs=5) as op,
            tc.tile_pool(name="ps", bufs=8, space="PSUM") as ps,
        ):
            kvTs = c1.tile([P, DPO, RKV], f32r)
            for po in range(DPO):
                nc.sync.dma_start(
                    kvTs[:, po],
                    kvTs_d.ap().rearrange("(po p) q -> po p q", p=P)[po])
            bk_all = c1.tile([P, DPO], f32)
            nc.sync.dma_start(bk_all[:], bk_d.ap())
            wk_r = wk_d.ap().rearrange("(po p) h -> p po h", p=P)
            wv_r = wv_d.ap().rearrange("(po p) h -> p po h", p=P)
            # prefetch all weight blocks up-front so the PE stream is dense
            wk_blks = []
            for hc in range(DPO):
                wkb = wkp.tile([P, DPO, P], f32r, tag="wk", name=f"wkb{hc}")
                nc.sync.dma_start(wkb[:], wk_r[:, :, P * hc:P * (hc + 1)])
                wk_blks.append(wkb)
            wv_blks = []
            for n in range(2):
                wvb = wvp.tile([P, DPO, 512], f32r, tag="wv", name=f"wvb{n}")
                nc.sync.dma_start(wvb[:], wv_r[:, :, 512 * n:512 * (n + 1)])
                wv_blks.append(wvb)

            # K^T po-outer: 8 parallel psum accumulators so the PE stream
            # is dense from the first kvTs chunk (keeps the p-state warm)
            ps_ks = [ps.tile([P, RKV], f32, tag="k", name=f"ps_k{_h}")
                     for _h in range(DPO)]
            for po in range(DPO):
                for hc in range(DPO):
                    nc.tensor.matmul(ps_ks[hc][:], wk_blks[hc][:, po],
                                     kvTs[:, po], start=(po == 0),
                                     stop=(po == DPO - 1))
            for hc in range(DPO):
                kt_o = op.tile([P, RKV], f32, tag="kt")
                nc.scalar.activation(kt_o[:], ps_ks[hc][:], AF.Identity,
                                     bias=bk_all[:, hc:hc + 1])
                nc.sync.dma_start(
                    ktp_d.ap().rearrange("(hc p) q -> hc p q", p=P)[hc], kt_o[:])

            # V: for each kv 128-chunk t, hd 512-chunk n
            for n in range(2):
                wv_blk = wv_blks[n]
                for t in range(RKV // P):
                    ps_v = ps.tile([P, RKV], f32, tag="k",
                                   name="ps_v")[:, :512]
                    for po in range(DPO):
                        nc.tensor.matmul(
                            ps_v[:], kvTs[:, po, P * t:P * (t + 1)],
                            wv_blk[:, po], start=(po == 0), stop=(po == DPO - 1))
                    v_o = op.tile([P, 512], f32, tag="v")
                    nc.vector.tensor_copy(v_o[:], ps_v[:])
                    nc.sync.dma_start(
                        vp_d.ap().rearrange("(t p) (n f) -> t n p f",
                                            p=P, f=512)[t, n], v_o[:])
    nc.compile()
    return nc


def build_phase2():
    nc = bacc.Bacc("TRN2", target_bir_lowering=False, debug=False,
                   num_devices=N_CORES)
    f32r, f32 = dt.float32r, dt.float32
    qT_d = nc.dram_tensor("qT", [DQ, RQ], f32r, kind="ExternalInput")
    kt_d = nc.dram_tensor("kt", [HID, LKV], f32r, kind="ExternalInput")
    va_d = nc.dram_tensor("va", [LKV, NH * VA], f32r, kind="ExternalInput")
    xq_d = nc.dram_tensor("xq", [RQ, HID], f32, kind="ExternalInput")
    wq_d = nc.dram_tensor("wq", [DQ, HID], f32r, kind="ExternalInput")
    wo_d = nc.dram_tensor("wo", [HID, DQ], f32r, kind="ExternalInput")
    bq_d = nc.dram_tensor("bq", [P, N_PAIR], f32, kind="ExternalInput")
    bv_d = nc.dram_tensor("bv", [HD, NH], f32, kind="ExternalInput")
    gam_d = nc.dram_tensor("gamma", [1, DQ], f32r, kind="ExternalInput")
    bet_d = nc.dram_tensor("beta", [1, DQ], f32r, kind="ExternalInput")
    out_d = nc.dram_tensor("out", [RQ, DQ], f32, kind="ExternalOutput")

    with tile.TileContext(nc) as tc:
        const_cm = tc.tile_pool(name="const", bufs=1)
        const = const_cm.__enter__()
        wq0 = const.tile([P, DPO, P], f32r)
        wq_r = wq_d.ap().rearrange("(po p) h -> p po h", p=P)
        nc.sync.dma_start(wq0[:], wq_r[:, :, 0:P])
        bq_all = const.tile([P, N_PAIR], f32)
        bv_all = const.tile([HD, NH], f32)
        nc.sync.dma_start(bq_all[:], bq_d.ap())
        nc.sync.dma_start(bv_all[:], bv_d.ap())
        qT_sb = const.tile([P, DPO, RQ], f32r)
        for po in range(DPO):
            nc.sync.dma_start(
                qT_sb[:, po], qT_d.ap().rearrange("(po p) q -> po p q", p=P)[po])
        eps_t = const.tile([P, 1], f32)
        nc.vector.memset(eps_t[:], EPS)
        gb_bc = const.tile([P, 2, DQ], f32)
        ctxT_sb = const.tile([P, N_PAIR, RQ], f32r)
        wo_sb = const.tile([P, DPO, DQ], f32r)

        kt_r = kt_d.ap().rearrange("(hp p) q -> hp p q", p=P)
        # va viewed [kvpo, p, quartet, 4*VA]
        va_r = va_d.ap().rearrange("(po p) (qt v) -> po p qt v", p=P, v=4 * VA)
        wo_r = wo_d.ap().rearrange("(po p) e -> po p e", p=P)

        with (
            tc.tile_pool(name="vpool", bufs=3) as vpool,
            tc.tile_pool(name="ktpool", bufs=3) as ktpool,
            tc.tile_pool(name="qtpool", bufs=3) as qtpool,
            tc.tile_pool(name="epool", bufs=5) as epool,
            tc.tile_pool(name="wpool", bufs=2) as wpool,
            tc.tile_pool(name="bpool", bufs=3) as bpool,
            tc.tile_pool(name="smpool", bufs=4) as smpool,
            tc.tile_pool(name="sc_ps", bufs=2, space="PSUM") as sc_ps,
            tc.tile_pool(name="ctx_ps", bufs=4, space="PSUM") as ctx_ps,
        ):
            def q_proj(hp, name):
                if hp == 0:
                    wq_blk = wq0
                else:
                    wq_blk = wpool.tile([P, DPO, P], f32r, tag="w",
                                        name=f"wqb{hp}")
                    nc.sync.dma_start(wq_blk[:],
                                      wq_r[:, :, P * hp:P * (hp + 1)])
                ps_q = ctx_ps.tile([P, RQ], f32, tag="ctx", name=f"psq{hp}")
                for po in range(DPO):
                    nc.tensor.matmul(ps_q[:], wq_blk[:, po], qT_sb[:, po],
                                     start=(po == 0), stop=(po == DPO - 1))
                qt_t = qtpool.tile([P, RQ], f32r, tag="qt", name=name)
                nc.vector.tensor_scalar(qt_t[:], ps_q[:],
                                        bq_all[:, hp:hp + 1], None, op0=ALU.add)
                return qt_t

            qt_next = None
            for hp in range(N_PAIR):
                # K^T for pair straight from DRAM
                kt_pair = ktpool.tile([P, LKV], f32r, tag="kt")
                for kc in range(4):
                    nc.sync.dma_start(kt_pair[:, 512 * kc:512 * (kc + 1)],
                                      kt_r[hp, :, 512 * kc:512 * (kc + 1)])
                nc.sync.dma_start(wo_sb[:, hp], wo_r[hp])
                if hp == 0:
                    for i, rd in enumerate((gam_d, bet_d)):
                        row = bpool.tile([1, DQ], f32r, tag="recbc",
                                         name=f"row{i}")
                        nc.sync.dma_start(row[:], rd.ap())
                        nc.gpsimd.partition_broadcast(gb_bc[:, i, :],
                                                      row[:].bitcast(f32))
                # V quartet from DRAM
                if hp % 2 == 0:
                    qt4 = hp // 2
                    v_sb = vpool.tile([P, KV_T, 4 * VA], f32r, tag="v")
                    nc.sync.dma_start(
                        v_sb[:],
                        va_r[:, :, qt4, :].rearrange("po p v -> p po v"))

                # Q^T projection (pair 0 inline; later pairs were hoisted)
                qt_pair = q_proj(0, "qt0") if hp == 0 else qt_next

                # attention
                ps_c = [ctx_ps.tile([VA, RQ], f32, tag="ctx", name=f"ps_c{_h}")
                        for _h in range(2)]
                for kv in range(KV_T):
                    ps_s = sc_ps.tile([P, 2, RQ], f32, tag="sc")
                    for h in range(2):
                        lo, hi = HD * h, HD * (h + 1)
                        nc.tensor.matmul(
                            ps_s[:, h], kt_pair[lo:hi, P * kv:P * (kv + 1)],
                            qt_pair[lo:hi, :], start=True, stop=True,
                            tile_position=(HD * h, 0))
                    e_t = epool.tile([P, 2, RQ], f32r, tag="e")
                    nc.scalar.activation(e_t[:], ps_s[:], AF.Exp,
                                         scale=1.0 / np.sqrt(HD))
                    for h in range(2):
                        hq = (hp % 2) * 2 + h
                        nc.tensor.matmul(
                            ps_c[h][:],
                            v_sb[:, kv, VA * hq:VA * (hq + 1)],
                            e_t[:, h], start=(kv == 0), stop=(kv == KV_T - 1))

                if hp < N_PAIR - 1:
                    qt_next = q_proj(hp + 1, f"qt{hp + 1}")

                # normalize + bv
                for h in range(2):
                    rec = smpool.tile([1, RQ], f32, tag="rec")
                    nc.vector.reciprocal(rec[:], ps_c[h][HD:HD + 1, :])
                    rec_bc = bpool.tile([HD, RQ], f32, tag="recbc")
                    nc.gpsimd.partition_broadcast(rec_bc[:], rec[:])
                    dst = ctxT_sb[HD * h:HD * (h + 1), hp, :]
                    nc.vector.tensor_tensor(dst, ps_c[h][:HD, :], rec_bc[:],
                                            op=ALU.mult)
                    nc.vector.tensor_scalar(
                        dst, dst, bv_all[:, 2 * hp + h:2 * hp + h + 1], None,
                        op0=ALU.add)

        # output projection + residual + LayerNorm
        with (
            tc.tile_pool(name="opool", bufs=2) as opool,
            tc.tile_pool(name="xqpool", bufs=4) as xqpool,
            tc.tile_pool(name="ln_sm", bufs=4) as ln_sm,
            tc.tile_pool(name="out_ps", bufs=4, space="PSUM") as out_ps,
        ):
            ps_os = [out_ps.tile([P, 2, 512], f32, tag="o", name=f"pso{_m}")
                     for _m in range(MQ)]
            xq_ts = []
            for m in range(MQ):
                xq_t = xqpool.tile([P, DQ], f32, tag="xq", name=f"xq{m}")
                nc.sync.dma_start(
                    xq_t[:], xq_d.ap().rearrange("(m p) e -> m p e", p=P)[m])
                xq_ts.append(xq_t)
            for m in range(MQ):
                for po in range(DPO):
                    for n in range(2):
                        nc.tensor.matmul(
                            ps_os[m][:, n], ctxT_sb[:, po, P * m:P * (m + 1)],
                            wo_sb[:, po, 512 * n:512 * (n + 1)],
                            start=(po == 0), stop=(po == DPO - 1))
                xq_t = xq_ts[m]
                x = opool.tile([P, DQ], f32, tag="x")
                mu = ln_sm.tile([P, 1], f32, tag="mu")
                nc.vector.scalar_tensor_tensor(
                    x[:], ps_os[m][:].rearrange("p a b -> p (a b)"), 1.0,
                    xq_t[:], op0=ALU.mult, op1=ALU.add, accum_out=mu[:])
                xx = opool.tile([P, DQ], f32, tag="xx")
                m2 = ln_sm.tile([P, 1], f32, tag="m2")
                nc.scalar.activation(xx[:], x[:], AF.Square, accum_out=m2[:])
                nc.vector.tensor_scalar(mu[:], mu[:], 1.0 / DQ, None,
                                        op0=ALU.mult)
                musq = ln_sm.tile([P, 1], f32, tag="musq")
                nc.vector.tensor_tensor(musq[:], mu[:], mu[:], op=ALU.mult)
                var = ln_sm.tile([P, 1], f32, tag="var")
                nc.vector.tensor_scalar(var[:], m2[:], 1.0 / DQ, None,
                                        op0=ALU.mult)
                nc.vector.tensor_tensor(var[:], var[:], musq[:],
                                        op=ALU.subtract)
                sd = ln_sm.tile([P, 1], f32, tag="sd")
                nc.scalar.activation(sd[:], var[:], AF.Sqrt, bias=eps_t[:])
                rstd = ln_sm.tile([P, 1], f32, tag="rstd")
                nc.vector.reciprocal(rstd[:], sd[:])
                y = opool.tile([P, DQ], f32, tag="xx")
                nc.vector.scalar_tensor_tensor(
                    y[:], x[:], mu[:], gb_bc[:, 0], op0=ALU.subtract,
                    op1=ALU.mult)
                z = opool.tile([P, DQ], f32, tag="x")
                nc.vector.tensor_scalar(z[:], y[:], rstd[:], None, op0=ALU.mult)
                z2 = opool.tile([P, DQ], f32, tag="xx")
                nc.gpsimd.tensor_tensor(z2[:], z[:], gb_bc[:, 1], op=ALU.add)
                nc.sync.dma_start(
                    out_d.ap().rearrange("(m p) e -> m p e", p=P)[m], z2[:])
        const_cm.__exit__(None, None, None)

    nc.compile()
    return nc


_CACHE = {}


def _get(name):
    if name not in _CACHE:
        _CACHE[name] = build_phase1() if name == "p1" else build_phase2()
    return _CACHE[name]


def kernel(query, key_value, Wq, bq, Wk, bk, Wv, bv, Wo, bo, ln_gamma, ln_beta):
    query = np.asarray(query, dtype=np.float32)
    key_value = np.asarray(key_value, dtype=np.float32)
    Wq = np.ascontiguousarray(np.asarray(Wq, np.float32))
    Wk = np.ascontiguousarray(np.asarray(Wk, np.float32))
    Wv = np.ascontiguousarray(np.asarray(Wv, np.float32))
    Wo = np.ascontiguousarray(np.asarray(Wo, np.float32))
    bq_a = np.ascontiguousarray(np.asarray(bq, np.float32).reshape(N_PAIR, P).T)
    bk_a = np.ascontiguousarray(np.asarray(bk, np.float32).reshape(DPO, P).T)
    bv_a = np.ascontiguousarray(np.asarray(bv, np.float32).reshape(NH, HD).T)
    gam = np.asarray(ln_gamma, np.float32).reshape(1, DQ)
    bet = np.asarray(ln_beta, np.float32).reshape(1, DQ)
    bo = np.asarray(bo, np.float32)

    # ---- phase 1: K^T / V projections, kv-sharded ----
    nc1 = _get("p1")
    kvT = [np.ascontiguousarray(key_value[b].T) for b in range(B)]
    in1 = []
    for c in range(N_CORES):
        b, rk = divmod(c, N_CORES // B)
        cols = slice(RKV * rk, RKV * (rk + 1))
        in1.append({
            "kvTs": np.ascontiguousarray(kvT[b][:, cols]),
            "wk": Wk, "wv": Wv, "bk": bk_a,
        })
    run_bass_kernel_spmd(nc1, in1, list(range(N_CORES)))
    r1 = run_bass_kernel_spmd(nc1, in1, list(range(N_CORES))).results

    kt_full = [np.concatenate([r1[4 * b + i]["ktp"] for i in range(4)], axis=1)
               for b in range(B)]
    v_full = [np.concatenate([r1[4 * b + i]["vp"] for i in range(4)], axis=0)
              for b in range(B)]
    va_full = []
    for b in range(B):
        va = np.ones((LKV, NH, VA), np.float32)
        va[:, :, :HD] = v_full[b].reshape(LKV, NH, HD)
        va_full.append(va.reshape(LKV, NH * VA))

    # ---- phase 2: attention ----
    nc2 = _get("p2")
    in2 = []
    for c in range(N_CORES):
        b, rq = divmod(c, N_CORES // B)
        rows = slice(RQ * rq, RQ * (rq + 1))
        in2.append({
            "qT": np.ascontiguousarray(query[b, rows].T),
            "kt": kt_full[b], "va": va_full[b],
            "xq": np.ascontiguousarray(query[b, rows] + bo),
            "wq": Wq, "wo": Wo, "bq": bq_a, "bv": bv_a,
            "gamma": gam, "beta": bet,
        })
    run_bass_kernel_spmd(nc2, in2, list(range(N_CORES)))
    res = run_bass_kernel_spmd(nc2, in2, list(range(N_CORES)))
    out = np.concatenate([r["out"] for r in res.results], axis=0)
    return out.reshape(B, LQ, DQ)



# revision 2
# speedup vs baseline: 1.0295x; 1.0295x over previous
"""Cross-attention layer (B=2, L=2048, D=1024, 16 heads) on 8 TRN2 NeuronCores.

fp8 DoubleRow rewrite of the baseline two-phase pipeline:

Phase 1 (kv-row sharded, core c -> batch c//4, kv rows 512*(c%4)..):
    K16^T = (16Wk)^T kvT   (fp8 DoubleRow, no bias: bk is softmax-invariant)
    V16   = kv (16Wv)      (fp8 DoubleRow, bv folded into bo on host)
Host gathers K/V, re-arranges into fp8 score/ctx layouts (free).

Phase 2 (q-row sharded): Q-projection (fp8 DR) + attention + out-projection
(fp8 DR) + residual LayerNorm. exp() is split between the ACT engine (true
exp -> fp8) and the DVE (Schraudolph bitcast exp -> int8 viewed as fp8);
softmax denominator comes from a ones-column appended to V. ctx is computed
in [q, hd] orientation, normalized+converted via reciprocal+broadcast
multiply, transposed on the PE (fp8, identity matmul) for the out-projection.

Scale plan: weights are pre-scaled x16 on the host so fp8 operands stay in
the normal range; scores psum = 256*QK ->  exp scale 1/(256*sqrt(64));
out-proj psum = 256*(ctx@Wo) -> LN x-pass multiplies by 1/256.
"""

import numpy as np
import ml_dtypes

import concourse.mybir as mybir
import concourse.tile as tile
from concourse import bacc
from concourse.bass_utils import run_bass_kernel_spmd

dt = mybir.dt
AF = mybir.ActivationFunctionType
ALU = mybir.AluOpType
PM = mybir.MatmulPerfMode
F8 = ml_dtypes.float8_e4m3
BF16 = ml_dtypes.bfloat16

P = 128
B, LQ, LKV = 2, 2048, 2048
DQ, DKV, HID, NH = 1024, 1024, 1024, 16
HD = HID // NH
EPS = 1e-5
N_CORES = 8
RQ = LQ * B // N_CORES             # 512 q rows per phase-2 core
RKV = LKV * B // N_CORES           # 512 kv rows per phase-1 core
WS = 16.0                          # host weight scale
SSC = 1.0 / (2 * WS * WS * np.sqrt(HD))  # scores psum (DR-doubled) -> true

# exp engine split: ACT gets ACT_FRAC of the 128 (head, kv-pair) exp tiles
# per-head exp schedules (alternating by head parity to balance ACT/DVE):
# ("A", [c0, c1]) = ACT pair-tile, ("D", [c]) = DVE single
EXP_SCHED_EVEN = [("A", [0, 1]), ("D", [2]), ("A", [3, 4]), ("D", [5]),
                  ("A", [6, 7]), ("D", [8]), ("A", [9, 10]), ("D", [11]),
                  ("A", [12, 13]), ("D", [14]), ("D", [15])]
EXP_SCHED_LIGHT = [("A", [0, 1]), ("D", [2]), ("D", [3]), ("A", [4, 5]),
                   ("D", [6]), ("D", [7]), ("A", [8, 9]), ("D", [10]),
                   ("D", [11]), ("A", [12, 13]), ("D", [14]), ("D", [15])]
# Schraudolph constants (int8 bits of fp8e4m3): bits = s*SH_A + SH_B
SH_A = float(8.0 * np.log2(np.e) * SSC)
SH_B = float(8.0 * (7.0 - 0.0432) + 0.5)


def build_phase1():
    nc = bacc.Bacc("TRN2", target_bir_lowering=False, debug=False,
                   num_devices=N_CORES)
    f32, f8 = dt.float32, dt.float8e4
    kvt_d = nc.dram_tensor("kvt", [P, 8 * RKV], f8, kind="ExternalInput")
    wk_d = nc.dram_tensor("wk", [P, 8 * HID], f8, kind="ExternalInput")
    wv_d = nc.dram_tensor("wv", [P, 8 * HID], f8, kind="ExternalInput")
    ktp_d = nc.dram_tensor("ktp", [P, 8 * RKV], f8, kind="ExternalOutput")
    vp_d = nc.dram_tensor("vp", [P, 4 * HID], f8, kind="ExternalOutput")

    with tile.TileContext(nc) as tc:
        with (
            tc.tile_pool(name="sb", bufs=1) as sb,
            tc.tile_pool(name="ps", bufs=1, space="PSUM") as ps,
        ):
            kvt = sb.tile([P, 4, 2, RKV], f8)
            nc.sync.dma_start(kvt[:], kvt_d.ap().rearrange(
                "p (j i q) -> p j i q", j=4, i=2))
            wk = sb.tile([P, 4, 2, HID], f8)
            wv = sb.tile([P, 4, 2, HID], f8)
            wk_r = wk_d.ap().rearrange("p (j i h) -> p j i h", j=4, i=2)
            wv_r = wv_d.ap().rearrange("p (j i h) -> p j i h", j=4, i=2)
            nc.sync.dma_start(wk[:, :, :, 0:512], wk_r[:, :, :, 0:512])
            nc.sync.dma_start(wk[:, :, :, 512:1024], wk_r[:, :, :, 512:1024])
            nc.sync.dma_start(wv[:, :, :, 0:512], wv_r[:, :, :, 0:512])
            nc.sync.dma_start(wv[:, :, :, 512:1024], wv_r[:, :, :, 512:1024])

            kt_st = sb.tile([P, 8, RKV], f8)
            v_st = sb.tile([P, 4, HID], f8)
            ktp_r = ktp_d.ap().rearrange("p (t q) -> p t q", q=RKV)
            vp_r = vp_d.ap().rearrange("p (t h) -> p t h", h=HID)

            # 8 pipeline tiles: K pairs (hid-tile pairs), then V (t-pair, hid
            # half). Conversions alternate ACT/DVE; last tile uses both.
            for k in range(4):
                ps_k = ps.tile([P, 2, RKV], f32, tag="big", bufs=4,
                               name=f"psk{k}")
                for sub in range(2):
                    ht = 2 * k + sub
                    for j in range(4):
                        nc.tensor.matmul(
                            ps_k[:, sub], wk[:, j, :, P * ht:P * (ht + 1)],
                            kvt[:, j], start=(j == 0), stop=(j == 3),
                            perf_mode=PM.DoubleRow)
                dst = kt_st[:, 2 * k:2 * k + 2]
                if k % 2 == 0:
                    nc.scalar.activation(dst, ps_k[:], AF.Copy)
                else:
                    nc.vector.tensor_copy(dst, ps_k[:])
                if k == 1:
                    nc.sync.dma_start(ktp_r[:, 0:4], kt_st[:, 0:4])
                if k == 3:
                    nc.sync.dma_start(ktp_r[:, 4:8], kt_st[:, 4:8])

            for k in range(4):
                tpair, n2 = k % 2, k // 2
                ps_v = ps.tile([P, 2, RKV], f32, tag="big", bufs=4,
                               name=f"psv{k}")
                for t2 in range(2):
                    t = 2 * tpair + t2
                    for j in range(4):
                        nc.tensor.matmul(
                            ps_v[:, t2], kvt[:, j, :, P * t:P * (t + 1)],
                            wv[:, j, :, 512 * n2:512 * (n2 + 1)],
                            start=(j == 0), stop=(j == 3),
                            perf_mode=PM.DoubleRow)
                dst = v_st[:, 2 * tpair:2 * tpair + 2,
                           512 * n2:512 * (n2 + 1)]
                if k == 3:
                    nc.scalar.activation(dst[:, 0], ps_v[:, 0], AF.Copy)
                    nc.vector.tensor_copy(dst[:, 1], ps_v[:, 1])
                elif k % 2 == 0:
                    nc.vector.tensor_copy(dst, ps_v[:])
                else:
                    nc.scalar.activation(dst, ps_v[:], AF.Copy)
                if k == 2:
                    nc.sync.dma_start(vp_r[:, :, 0:512], v_st[:, :, 0:512])
                if k == 3:
                    nc.sync.dma_start(vp_r[:, :, 512:1024],
                                      v_st[:, :, 512:1024])
    nc.compile()
    return nc


def build_phase2(ln_trivial):
    nc = bacc.Bacc("TRN2", target_bir_lowering=False, debug=False,
                   num_devices=N_CORES)
    f32, f8, bf = dt.float32, dt.float8e4, dt.bfloat16
    id_d = nc.dram_tensor("ident", [P, P], f8, kind="ExternalInput")
    bq_d = nc.dram_tensor("bq", [P, 8], f32, kind="ExternalInput")
    gam_d = nc.dram_tensor("gamma", [1, DQ], f32, kind="ExternalInput")
    bet_d = nc.dram_tensor("beta", [1, DQ], f32, kind="ExternalInput")
    qt_d = nc.dram_tensor("qt", [P, 8 * RQ], f8, kind="ExternalInput")
    wq_d = nc.dram_tensor("wq", [P, 8 * HID], f8, kind="ExternalInput")
    kt_d = nc.dram_tensor("kt", [P, 8 * 2 * LKV], f8, kind="ExternalInput")
    va_d = nc.dram_tensor("va", [P, 8 * 2 * NH * 65], f8, kind="ExternalInput")
    wo_d = nc.dram_tensor("wo", [P, 8 * DQ], f8, kind="ExternalInput")
    xq_d = nc.dram_tensor("xq", [P, 4 * DQ], bf, kind="ExternalInput")
    out_d = nc.dram_tensor("out", [P, 4 * DQ], bf, kind="ExternalOutput")

    kt_r = kt_d.ap().rearrange("p (t i k) -> p t i k", t=8, i=2)
    va_r = va_d.ap().rearrange("p (u i h c) -> p u i h c", u=8, i=2, h=NH)
    wq_r = wq_d.ap().rearrange("p (j i h) -> p j i h", j=4, i=2)
    wo_r = wo_d.ap().rearrange("p (j i e) -> p j i e", j=4, i=2)

    with tile.TileContext(nc) as tc:
        with (
            tc.tile_pool(name="sb", bufs=1) as sb,
            tc.tile_pool(name="ep", bufs=1) as ep,
            tc.tile_pool(name="ln", bufs=1) as ln,
        ):
            ident = sb.tile([P, P], f8)
            nc.sync.dma_start(ident[:], id_d.ap())
            bq_sb = sb.tile([P, 8], f32)
            nc.sync.dma_start(bq_sb[:], bq_d.ap())
            eps_t = sb.tile([P, 1], f32)
            nc.vector.memset(eps_t[:], EPS)
            gb_bc = None
            if not ln_trivial:
                gb_bc = sb.tile([P, 2, DQ], f32)
                for i, rd in enumerate((gam_d, bet_d)):
                    row = sb.tile([1, DQ], f32, name=f"gbrow{i}")
                    nc.sync.dma_start(row[:], rd.ap())
                    nc.gpsimd.partition_broadcast(gb_bc[:, i, :], row[:])

            qt_sb = sb.tile([P, 4, 2, RQ], f8)
            wq_sb = sb.tile([P, 4, 2, HID], f8)
            kt_sb = sb.tile([P, 8, 2, LKV], f8)
            va_sb = sb.tile([P, 8, 2, NH, 65], f8)
            wo_sb = sb.tile([P, 4, 2, DQ], f8)
            xq_sb = sb.tile([P, 4, DQ], bf)
            q16_sb = sb.tile([P, 8, RQ], f8)
            ctxn_sb = sb.tile([P, 4, NH, HD], f8)
            ctxT_sb = sb.tile([P, 4, 2, RQ], f8)

            # DMA schedule: earliest-needed first, batched
            nc.sync.dma_start(qt_sb[:], qt_d.ap().rearrange(
                "p (j i q) -> p j i q", j=4, i=2))
            nc.sync.dma_start(wq_sb[:], wq_r)
            nc.sync.dma_start(kt_sb[:, 0], kt_r[:, 0])
            nc.sync.dma_start(va_sb[:, 0:2], va_r[:, 0:2])
            nc.sync.dma_start(kt_sb[:, 1], kt_r[:, 1])
            nc.sync.dma_start(va_sb[:, 2:4], va_r[:, 2:4])
            nc.sync.dma_start(kt_sb[:, 2], kt_r[:, 2])
            nc.sync.dma_start(va_sb[:, 4:6], va_r[:, 4:6])
            nc.sync.dma_start(kt_sb[:, 3], kt_r[:, 3])
            nc.sync.dma_start(va_sb[:, 6:8], va_r[:, 6:8])
            nc.sync.dma_start(kt_sb[:, 4:6], kt_r[:, 4:6])
            nc.sync.dma_start(kt_sb[:, 6:8], kt_r[:, 6:8])
            nc.sync.dma_start(wo_sb[:], wo_r)
            nc.sync.dma_start(xq_sb[:], xq_d.ap().rearrange(
                "p (m e) -> p m e", m=4))

            with (
                tc.tile_pool(name="apool", bufs=1, space="PSUM") as apool,
                tc.tile_pool(name="dpool", bufs=1, space="PSUM") as dpool,
                tc.tile_pool(name="cpool", bufs=1, space="PSUM") as cpool,
            ):
                def q_proj(ht):
                    on_act = ht % 2 == 0
                    if on_act:
                        ps_q = apool.tile([P, 2, RQ], f32, tag="scA", bufs=2,
                                          name=f"psq{ht}")[:, 0]
                    else:
                        ps_q = dpool.tile([P, RQ], f32, tag="scD", bufs=3,
                                          name=f"psq{ht}")
                    for j in range(4):
                        nc.tensor.matmul(ps_q[:],
                                         wq_sb[:, j, :, P * ht:P * (ht + 1)],
                                         qt_sb[:, j], start=(j == 0),
                                         stop=(j == 3), perf_mode=PM.DoubleRow)
                    if on_act:
                        nc.scalar.activation(q16_sb[:, ht], ps_q[:],
                                             AF.Identity,
                                             bias=bq_sb[:, ht:ht + 1])
                    else:
                        nc.vector.tensor_scalar(
                            q16_sb[:, ht], ps_q[:], bq_sb[:, ht:ht + 1],
                            None, op0=ALU.add)

                e_tiles = {}
                ctx_ps = [None, None]

                def ctx_mm(h, u):
                    eh = e_tiles[h % 2]
                    cps = ctx_ps[h % 2]
                    for m in range(4):
                        nc.tensor.matmul(
                            cps[:, m], eh[:, 2 * u:2 * u + 2, P * m:P * (m + 1)],
                            va_sb[:, u, :, h, :],
                            start=(u == 0 and m == 0),
                            stop=(u == 7 and m == 3),
                            perf_mode=PM.DoubleRow, skip_group_check=True)

                def finish_head(h):
                    cps = ctx_ps[h % 2]
                    rec = ln.tile([P, 4], f32, tag="rec", bufs=2,
                                  name=f"rec{h}")
                    nc.vector.reciprocal(rec[:], cps[:, :, 64])
                    nc.vector.tensor_tensor(
                        ctxn_sb[:, :, h, :], cps[:, :, 0:64],
                        rec[:, :, None].broadcast_to([P, 4, HD]), op=ALU.mult)
                    if h % 2 == 1:
                        g = h // 2
                        tp = dpool.tile([P, 4, P, 2], f8, tag="scD", bufs=3,
                                        name=f"tp{g}")
                        for m in range(4):
                            nc.tensor.transpose(
                                tp[:, m, :, 0],
                                ctxn_sb[:, m, h - 1:h + 1, :], ident[:])
                        nc.vector.tensor_copy(
                            ctxT_sb[:, g // 2, g % 2, :], tp[:, :, :, 0])

                for h in range(NH + 1):
                    if h < NH:
                        if h == 0:
                            q_proj(0)
                            q_proj(1)
                        elif h <= 6:
                            q_proj(h + 1)
                        ctx_ps[h % 2] = cpool.tile(
                            [P, 4, 65], f32, tag="ctx", bufs=1,
                            padded_shape=[P, 4, P], name=f"ctx{h}")
                        e_tiles[h % 2] = ep.tile([P, 16, RQ], f8, tag="e",
                                                 bufs=2, name=f"e{h}")
                        b64, t = 64 * (h % 2), h // 2
                        rhs_bc = q16_sb[b64:b64 + 64, t, None, :]\
                            .broadcast_to([64, 2, RQ])
                        e_cur = e_tiles[h % 2]
                        ndone = 0
                        nctx = 0
                        sched = EXP_SCHED_LIGHT if h in (5, 11) \
                            else EXP_SCHED_EVEN
                        for eng, cs in sched:
                            if eng == "A":
                                sc = apool.tile([P, 2, RQ], f32, tag="scA",
                                                bufs=2, name=f"sc{h}_{cs[0]}")
                                for k, c in enumerate(cs):
                                    nc.tensor.matmul(
                                        sc[:, k],
                                        kt_sb[b64:b64 + 64, t, :,
                                              P * c:P * (c + 1)],
                                        rhs_bc, start=True, stop=True,
                                        perf_mode=PM.DoubleRow,
                                        tile_position=(b64, 0))
                                nc.scalar.activation(
                                    e_cur[:, cs[0]:cs[0] + 2, :], sc[:],
                                    AF.Exp, scale=SSC)
                            else:
                                c = cs[0]
                                sc = dpool.tile([P, RQ], f32, tag="scD",
                                                bufs=3, name=f"sc{h}_{c}")
                                nc.tensor.matmul(
                                    sc[:],
                                    kt_sb[b64:b64 + 64, t, :,
                                          P * c:P * (c + 1)],
                                    rhs_bc, start=True, stop=True,
                                    perf_mode=PM.DoubleRow,
                                    tile_position=(b64, 0))
                                nc.vector.tensor_scalar(
                                    e_cur[:, c, :].bitcast(dt.int8), sc[:],
                                    SH_A, SH_B, op0=ALU.mult, op1=ALU.add)
                            ndone += len(cs)
                            if h > 0 and ndone >= 2:
                                while nctx < 8:
                                    ctx_mm(h - 1, nctx)
                                    nctx += 1
                            if h > 0 and nctx == 8 and ndone in (4, 5):
                                finish_head(h - 1)
                                nctx = 9
                    else:
                        for u in range(8):
                            ctx_mm(NH - 1, u)
                        finish_head(NH - 1)

            # ---- tail: out-projection + residual LayerNorm ----
            with tc.tile_pool(name="opool", bufs=1, space="PSUM") as opool:
                for m in range(4):
                    ps_o = opool.tile([P, DQ], f32, tag="o", bufs=2,
                                      name=f"pso{m}")
                    for n in range(2):
                        for j in range(4):
                            nc.tensor.matmul(
                                ps_o[:, 512 * n:512 * (n + 1)],
                                ctxT_sb[:, j, :, P * m:P * (m + 1)],
                                wo_sb[:, j, :, 512 * n:512 * (n + 1)],
                                start=(j == 0), stop=(j == 3),
                                perf_mode=PM.DoubleRow)
                    x = ln.tile([P, DQ], f32, tag="x", bufs=2, name=f"x{m}")
                    sx = ln.tile([P, 1], f32, tag="sx", bufs=2, name=f"sx{m}")
                    nc.vector.scalar_tensor_tensor(
                        x[:], ps_o[:], 1.0 / (WS * WS), xq_sb[:, m],
                        op0=ALU.mult, op1=ALU.add, accum_out=sx[:])
                    xx = ln.tile([P, DQ], f32, tag="xx", bufs=2, name=f"xx{m}")
                    s2 = ln.tile([P, 1], f32, tag="s2", bufs=2, name=f"s2{m}")
                    nc.scalar.activation(xx[:], x[:], AF.Square,
                                         accum_out=s2[:])
                    negmu = ln.tile([P, 1], f32, tag="negmu", bufs=2,
                                    name=f"nmu{m}")
                    nc.vector.tensor_scalar(negmu[:], sx[:], -1.0 / DQ, None,
                                            op0=ALU.mult)
                    var = ln.tile([P, 1], f32, tag="var", bufs=2,
                                  name=f"var{m}")
                    nc.vector.tensor_scalar(var[:], s2[:], 1.0 / DQ, None,
                                            op0=ALU.mult)
                    musq = ln.tile([P, 1], f32, tag="musq", bufs=2,
                                   name=f"musq{m}")
                    nc.vector.tensor_tensor(musq[:], negmu[:], negmu[:],
                                            op=ALU.mult)
                    nc.vector.tensor_tensor(var[:], var[:], musq[:],
                                            op=ALU.subtract)
                    sd = ln.tile([P, 1], f32, tag="sd", bufs=2, name=f"sd{m}")
                    nc.scalar.activation(sd[:], var[:], AF.Sqrt, bias=eps_t[:])
                    rstd = ln.tile([P, 1], f32, tag="rstd", bufs=2,
                                   name=f"rstd{m}")
                    nc.vector.reciprocal(rstd[:], sd[:])
                    y = ln.tile([P, DQ], bf, tag="y", bufs=2, name=f"y{m}")
                    nc.scalar.activation(y[:], x[:], AF.Identity,
                                         bias=negmu[:])
                    if m == 0:
                        z_all = ln.tile([P, 4, DQ], bf, tag="z", bufs=1,
                                        name="z_all")
                    if ln_trivial:
                        nc.vector.tensor_scalar(z_all[:, m], y[:], rstd[:],
                                                None, op0=ALU.mult)
                    else:
                        z1 = ln.tile([P, DQ], f32, tag="z1", bufs=2,
                                     name=f"z1{m}")
                        nc.vector.scalar_tensor_tensor(
                            z1[:], y[:], rstd[:], gb_bc[:, 0], op0=ALU.mult,
                            op1=ALU.mult)
                        nc.gpsimd.tensor_tensor(z_all[:, m], z1[:],
                                                gb_bc[:, 1], op=ALU.add)
                if True:
                    nc.sync.dma_start(
                        out_d.ap().rearrange("p (m e) -> p m e", m=4),
                        z_all[:])
    nc.compile()
    return nc


_CACHE = {}


def _get(name):
    if name not in _CACHE:
        if name == "p1":
            _CACHE[name] = build_phase1()
        elif name == "p2":
            _CACHE[name] = build_phase2(True)
        else:
            _CACHE[name] = build_phase2(False)
    return _CACHE[name]


def _dr_rows(w):
    """[dk, X] -> [128, 4, 2, X] DoubleRow row layout, flattened [128, 4*2*X]."""
    x = w.shape[1]
    return np.ascontiguousarray(
        w.reshape(4, 2, 128, x).transpose(2, 0, 1, 3).reshape(128, 8 * x))


def kernel(query, key_value, Wq, bq, Wk, bk, Wv, bv, Wo, bo, ln_gamma, ln_beta):
    query = np.asarray(query, np.float32)
    key_value = np.asarray(key_value, np.float32)
    Wq = np.asarray(Wq, np.float32)
    Wk = np.asarray(Wk, np.float32)
    Wv = np.asarray(Wv, np.float32)
    Wo = np.asarray(Wo, np.float32)
    bq = np.asarray(bq, np.float32)
    bv = np.asarray(bv, np.float32)
    bo = np.asarray(bo, np.float32)
    gam = np.asarray(ln_gamma, np.float32)
    bet = np.asarray(ln_beta, np.float32)

    ln_trivial = bool(np.all(gam == 1.0) and np.all(bet == 0.0))

    # fold bv into the output bias (softmax weights sum to 1)
    bo_eff = bo + (bv.reshape(NH, HD)[:, :, None]
                   * Wo.reshape(NH, HD, DQ)).sum((0, 1))

    wk_l = _dr_rows((WS * Wk).astype(F8).astype(F8))
    wv_l = _dr_rows((WS * Wv).astype(F8))
    wo_l = _dr_rows((WS * Wo).astype(F8))

    wq_l = _dr_rows((WS * Wq).astype(F8))
    bq_l = np.ascontiguousarray((WS * bq).astype(np.float32)
                                .reshape(8, 128).T)

    ident = np.eye(P, dtype=F8)

    # ---- phase 1 ----
    nc1 = _get("p1")
    in1 = []
    for c in range(N_CORES):
        b, rk = divmod(c, N_CORES // B)
        rows = slice(RKV * rk, RKV * (rk + 1))
        kvt = key_value[b, rows].T.astype(F8)            # [1024, 512]
        kvt_l = np.ascontiguousarray(
            kvt.reshape(4, 2, 128, RKV).transpose(2, 0, 1, 3)
            .reshape(128, 8 * RKV))
        in1.append({"kvt": kvt_l, "wk": wk_l, "wv": wv_l})
    r1 = run_bass_kernel_spmd(nc1, in1, list(range(N_CORES))).results

    kts, vs = [], []
    for b in range(B):
        kt_parts = [np.asarray(r1[4 * b + i]["ktp"]).reshape(128, 8, RKV)
                    .transpose(1, 0, 2).reshape(HID, RKV) for i in range(4)]
        kts.append(np.concatenate(kt_parts, axis=1))     # K16^T [1024, 2048]
        v_parts = [np.asarray(r1[4 * b + i]["vp"]).reshape(128, 4, HID)
                   .transpose(1, 0, 2).reshape(RKV, HID) for i in range(4)]
        vs.append(np.concatenate(v_parts, axis=0))       # V16 [2048, 1024]

    kt_ls, va_ls = [], []
    for b in range(B):
        K3 = kts[b].reshape(8, 2, 64, LKV)               # [t, u, r, kv]
        kt_arr = K3.transpose(1, 2, 0, 3).reshape(128, 8, 1, LKV)
        kt_l = np.ascontiguousarray(
            np.broadcast_to(kt_arr, (128, 8, 2, LKV)).reshape(128, -1))
        kt_ls.append(kt_l)
        V6 = vs[b].astype(np.float32).reshape(8, 2, 128, NH, HD)
        va = np.ones((128, 8, 2, NH, 65), np.float32)
        va[:, :, :, :, :HD] = V6.transpose(2, 0, 1, 3, 4)
        va_ls.append(np.ascontiguousarray(va.astype(F8).reshape(128, -1)))

    # ---- phase 2 ----
    nc2 = _get("p2" if ln_trivial else "p2g")
    in2 = []
    for c in range(N_CORES):
        b, rq = divmod(c, N_CORES // B)
        rows = slice(RQ * rq, RQ * (rq + 1))
        qt = query[b, rows].T.astype(F8)                 # [1024, 512]
        qt_l = np.ascontiguousarray(
            qt.reshape(4, 2, 128, RQ).transpose(2, 0, 1, 3).reshape(128, -1))
        xq = (query[b, rows] + bo_eff).astype(BF16)      # [512, 1024]
        xq_l = np.ascontiguousarray(
            xq.reshape(4, 128, DQ).transpose(1, 0, 2).reshape(128, -1))
        in2.append({
            "ident": ident, "bq": bq_l,
            "gamma": gam.reshape(1, DQ), "beta": bet.reshape(1, DQ),
            "qt": qt_l, "wq": wq_l, "kt": kt_ls[b], "va": va_ls[b],
            "wo": wo_l, "xq": xq_l,
        })
    res = run_bass_kernel_spmd(nc2, in2, list(range(N_CORES))).results

    out = np.empty((B, LQ, DQ), np.float32)
    for c in range(N_CORES):
        b, rq = divmod(c, N_CORES // B)
        o = np.asarray(res[c]["out"]).astype(np.float32)
        out[b, RQ * rq:RQ * (rq + 1)] = (
            o.reshape(128, 4, DQ).transpose(1, 0, 2).reshape(RQ, DQ))
    return out
